# revision 15
# baseline (speedup 1.0000x reference)
"""Sharded GAT graph classifier for Trainium2 (Bass/Tile, 8 NeuronCores).

Strategy (1D dst-partitioning, per the sharding hint):
 - nodes (message destinations) sharded across 8 cores; each core owns all
   edges whose dst falls in its shard (self-loops included as edges),
   sorted/grouped into 128-dst windows
 - per-core: layer-1 node table (BN-folded projected features + src attention
   logits, fp16) rebuilt locally, then the edge phase gathers source rows with
   one indirect DMA per 128-edge block and aggregates messages with one-hot
   segment-sum matmuls into PSUM windows; dst logits are expanded per edge
   with a one-hot-transpose matmul against the window's a_d chunk
 - layer-2 table shards exchanged with one AllGather; pooling partials
   combined with one AllReduce; classifier replicated
 - softmax uses exp without max-subtraction (logits bounded ~|4.3|, fp32 exp)
 - BN folded into table scales; ELU carried as elu(x)+1 with the -1 folded
   into downstream bias columns
"""
import math
import numpy as np
import ml_dtypes  # noqa: F401
from contextlib import ExitStack

import concourse.bass as bass
import concourse.tile as tile
from concourse import bacc, mybir
from concourse.bass_utils import run_bass_kernel_spmd

F16 = mybir.dt.float16
F32 = mybir.dt.float32
I32 = mybir.dt.int32
AF = mybir.ActivationFunctionType
ALU = mybir.AluOpType

BN_EPS = 1e-5
SLOPE = 0.2

G = 512
IN = 64
H = 4
HID = 32
OUT = 2
HH = H * HID            # 128
C1 = HH + H             # 132 f16 units per table1 row: [xs' x128 | a_s x4]
TW2 = HID + 4           # 36 f16 units per table2 row: [xs2' x32 | a_s2 | a_d2 | 0 0]
NC = 8

KB = 32                 # edge blocks per chunk
CT = 16                 # node tiles per cast chunk in table build


# --------------------------------------------------------------------------
# host-side index/layout preprocessing (no FP math on data)
# --------------------------------------------------------------------------
def _prep(edge_index, batch, N, n_cores):
    SH = N // n_cores
    W = (SH + 127) // 128
    NP = W * 128
    NT = (N + 127) // 128
    NPAD = NT * 128
    assert n_cores * SH == N

    ar = np.arange(N, dtype=np.int64)
    src = np.concatenate([edge_index[0].astype(np.int64), ar])
    dst = np.concatenate([edge_index[1].astype(np.int64), ar])
    E = src.shape[0]

    core = dst // SH
    loc = dst % SH
    win = loc // 128
    dstloc = loc % 128

    cnt = np.zeros((n_cores, W), dtype=np.int64)
    np.add.at(cnt, (core, win), 1)
    blocks_w = np.maximum((cnt.max(axis=0) + 127) // 128, 1)
    NB = int(blocks_w.sum())
    L = NB * 128

    order = np.lexsort((win, core))
    s_src, s_core, s_win = src[order], core[order], win[order]
    s_dstloc = dstloc[order]

    wstart = np.zeros(W, dtype=np.int64)
    wstart[1:] = np.cumsum(blocks_w * 128)[:-1]

    src_idx = np.zeros((n_cores, L), dtype=np.int64)
    dl = np.full((n_cores, L), 300, dtype=np.int64)
    grp = s_core * W + s_win
    grp_start = np.zeros(n_cores * W + 1, dtype=np.int64)
    np.add.at(grp_start, grp + 1, 1)
    grp_start = np.cumsum(grp_start)
    pos = np.arange(E) - grp_start[grp]
    slot = wstart[s_win] + pos
    src_idx[s_core, slot] = s_src
    dl[s_core, slot] = s_dstloc

    blk_win = np.repeat(np.arange(W), blocks_w)

    def pmaj(a, dt):
        return np.ascontiguousarray(
            a.reshape(n_cores, NB, 128).transpose(0, 2, 1)).astype(dt)

    srcP = pmaj(src_idx, np.int32)
    dlP = pmaj(dl, np.float16)
    dlF = np.ascontiguousarray(dl.astype(np.float16)).reshape(n_cores, 1, L)
    src2_idx = (src_idx // SH) * NP + (src_idx % SH)
    src2P = pmaj(src2_idx, np.int32)

    gl = np.full((n_cores, NP), 60000, dtype=np.int64)
    for c in range(n_cores):
        gl[c, :SH] = batch[c * SH:(c + 1) * SH]
    glP = np.ascontiguousarray(
        gl.reshape(n_cores, W, 128).transpose(0, 2, 1)).astype(np.float16)

    return dict(SH=SH, W=W, NP=NP, NT=NT, NPAD=NPAD, NB=NB,
                blocks_w=blocks_w, blk_win=blk_win,
                srcP=srcP, dlP=dlP, dlF=dlF, src2P=src2P, glP=glP)


def _bcast(ap, pattern):
    """AP on ap's tensor with an explicit [step, count] list."""
    return bass.AP(ap.tensor, ap.offset, [list(x) for x in pattern])


# --------------------------------------------------------------------------
# device program
# --------------------------------------------------------------------------
def _build(meta):
    W, NB, NPAD, NP = meta["W"], meta["NB"], meta["NPAD"], meta["NP"]
    NT = meta["NT"]
    blk_win = meta["blk_win"]
    n_cores = meta["n_cores"]

    wb0 = np.zeros(W, dtype=np.int64)
    wbl = np.zeros(W, dtype=np.int64)
    for w in range(W):
        idxs = np.where(blk_win == w)[0]
        wb0[w], wbl[w] = idxs[0], idxs[-1]

    nc = bacc.Bacc("TRN2", num_devices=n_cores)

    def dinp(name, shape, dt):
        return nc.dram_tensor(name, shape, dt, kind="ExternalInput")

    xT = dinp("xT", [IN, NPAD], F32)
    xT2 = dinp("xT2", [IN, NP], F32)
    W1 = dinp("W1", [IN, HH], F32)
    as1 = dinp("as1", [H, HID], F32)
    ad1 = dinp("ad1", [H, HID], F32)
    g1 = dinp("g1", [1, HH], F32); be1 = dinp("be1", [1, HH], F32)
    rm1 = dinp("rm1", [1, HH], F32); rv1 = dinp("rv1", [1, HH], F32)
    bb1 = dinp("bb1", [1, HH], F32)
    W2 = dinp("W2", [HH, HID], F32)
    as2 = dinp("as2", [1, HID], F32)
    ad2 = dinp("ad2", [1, HID], F32)
    g2 = dinp("g2", [1, HID], F32); be2 = dinp("be2", [1, HID], F32)
    rm2 = dinp("rm2", [1, HID], F32); rv2 = dinp("rv2", [1, HID], F32)
    bb2 = dinp("bb2", [1, HID], F32)
    Wc = dinp("Wc", [HID, OUT], F32)
    WcT = dinp("WcT", [OUT, HID], F32)
    bcT = dinp("bcT", [OUT, 1], F32)
    srcP = dinp("srcP", [128, NB], I32)
    src2P = dinp("src2P", [128, NB], I32)
    dlP = dinp("dlP", [128, NB], F16)
    dlF = dinp("dlF", [1, NB * 128], F16)
    glP = dinp("glP", [128, W], F16)
    iota128 = dinp("iota128", [1, 128], F16)
    iotaG = dinp("iotaG", [1, G], F16)
    iotaPf = dinp("iotaPf", [128, 1], F32)

    table1 = nc.dram_tensor("table1", [NPAD, C1], F16)
    t2in = nc.dram_tensor("t2in", [NP, TW2], F16)
    table2 = nc.dram_tensor("table2", [n_cores * NP, TW2], F16, addr_space="Shared")
    poolin = nc.dram_tensor("poolin", [HID + 1, G], F32)
    poolout = nc.dram_tensor("poolout", [HID + 1, G], F32, addr_space="Shared")
    b0d = nc.dram_tensor("b0d", [1, HH], F32)
    b2d = nc.dram_tensor("b2d", [1, HID], F32)
    a0d = nc.dram_tensor("a0d", [1, HH], F32)
    a2d = nc.dram_tensor("a2d", [1, HID], F32)
    cs2d = nc.dram_tensor("cs2d", [1, TW2], F32)
    recd = nc.dram_tensor("recd", [1, G], F32)
    nmd = nc.dram_tensor("nmd", [1, G], F32)
    outT = nc.dram_tensor("outT", [OUT, G], F32, kind="ExternalOutput")

    rg = [list(range(n_cores))]

    with ExitStack() as ctx:
        tc = ctx.enter_context(tile.TileContext(nc))
        cp = ctx.enter_context(tc.tile_pool(name="consts", bufs=1))

        # ============ phase 0: constants ============
        w1sb = cp.tile([IN, HH], F32)
        nc.sync.dma_start(w1sb[:], W1[:, :])
        w2sb = cp.tile([HH, HID], F32)
        nc.sync.dma_start(w2sb[:], W2[:, :])
        rows1 = {}
        for nm, t in [("g1", g1), ("be1", be1), ("rm1", rm1), ("rv1", rv1), ("bb1", bb1)]:
            rows1[nm] = cp.tile([1, HH], F32, name="r" + nm, tag="r" + nm)
            nc.sync.dma_start(rows1[nm][:], t[:, :])
        rows2 = {}
        for nm, t in [("g2", g2), ("be2", be2), ("rm2", rm2), ("rv2", rv2), ("bb2", bb2)]:
            rows2[nm] = cp.tile([1, HID], F32, name="r" + nm, tag="r" + nm)
            nc.sync.dma_start(rows2[nm][:], t[:, :])
        wcsb = cp.tile([HID, OUT], F32); nc.sync.dma_start(wcsb[:], Wc[:, :])
        wctsb = cp.tile([OUT, HID], F32); nc.sync.dma_start(wctsb[:], WcT[:, :])
        bctsb = cp.tile([OUT, 1], F32); nc.sync.dma_start(bctsb[:], bcT[:, :])
        glsb = cp.tile([128, W], F16); nc.sync.dma_start(glsb[:], glP[:, :])
        iotaPsb = cp.tile([128, 1], F32)
        nc.sync.dma_start(iotaPsb[:], iotaPf[:, :])
        # partition-replicated constants (HWDGE broadcast reads)
        iotaR = cp.tile([128, 128], F16)
        nc.sync.dma_start(out=iotaR[:], in_=_bcast(iota128[:, :], [[0, 128], [1, 128]]))
        iotaGR = cp.tile([128, G], F16)
        nc.sync.dma_start(out=iotaGR[:], in_=_bcast(iotaG[:, :], [[0, 128], [1, G]]))
        as1bR = cp.tile([IN, HH], F32)
        nc.sync.dma_start(out=as1bR[:], in_=_bcast(as1[:, :], [[0, IN], [1, HH]]))
        ad1bR = cp.tile([IN, HH], F32)
        nc.sync.dma_start(out=ad1bR[:], in_=_bcast(ad1[:, :], [[0, IN], [1, HH]]))
        as2bR = cp.tile([HH, HID], F32)
        nc.sync.dma_start(out=as2bR[:], in_=_bcast(as2[:, :], [[0, HH], [1, HID]]))
        ad2bR = cp.tile([HH, HID], F32)
        nc.sync.dma_start(out=ad2bR[:], in_=_bcast(ad2[:, :], [[0, HH], [1, HID]]))

        def bn_fold(rows, width, ad_, bd_):
            sd = cp.tile([1, width], F32, name="sd%d" % width, tag="sd%d" % width)
            nc.vector.tensor_scalar_add(sd[:], rows["rv"][:], BN_EPS)
            nc.scalar.activation(sd[:], sd[:], AF.Sqrt)
            rs = cp.tile([1, width], F32, name="rs%d" % width, tag="rs%d" % width)
            nc.vector.reciprocal(rs[:], sd[:])
            A = cp.tile([1, width], F32, name="A%d" % width, tag="A%d" % width)
            nc.vector.tensor_mul(A[:], rs[:], rows["g"][:])
            B = cp.tile([1, width], F32, name="B%d" % width, tag="B%d" % width)
            nc.vector.tensor_sub(B[:], rows["bb"][:], rows["rm"][:])
            nc.vector.tensor_mul(B[:], B[:], A[:])
            nc.vector.tensor_add(B[:], B[:], rows["be"][:])
            da = nc.sync.dma_start(ad_[:, :], A[:])
            db = nc.sync.dma_start(bd_[:, :], B[:])
            return da, db

        da0, db0 = bn_fold({"rv": rows1["rv1"], "g": rows1["g1"], "bb": rows1["bb1"],
                            "rm": rows1["rm1"], "be": rows1["be1"]}, HH, a0d, b0d)
        da2, db2 = bn_fold({"rv": rows2["rv2"], "g": rows2["g2"], "bb": rows2["bb2"],
                            "rm": rows2["rm2"], "be": rows2["be2"]}, HID, a2d, b2d)
        A0R = cp.tile([IN, HH], F32)
        d = nc.sync.dma_start(out=A0R[:], in_=_bcast(a0d[:, :], [[0, IN], [1, HH]]))
        tile.add_dep_helper(d.ins, da0.ins, sync=True)
        B0R = cp.tile([128, HH], F32)
        d = nc.sync.dma_start(out=B0R[:], in_=_bcast(b0d[:, :], [[0, 128], [1, HH]]))
        tile.add_dep_helper(d.ins, db0.ins, sync=True)
        A2R = cp.tile([HH, HID], F32)
        d = nc.sync.dma_start(out=A2R[:], in_=_bcast(a2d[:, :], [[0, HH], [1, HID]]))
        tile.add_dep_helper(d.ins, da2.ins, sync=True)
        B2R = cp.tile([128, HID], F32)
        d = nc.sync.dma_start(out=B2R[:], in_=_bcast(b2d[:, :], [[0, 128], [1, HID]]))
        tile.add_dep_helper(d.ins, db2.ins, sync=True)

        # W1ext [IN, 136] = [W1*A0 | w_as1 | w_ad1]
        w1e = cp.tile([IN, HH + 2 * H], F32)
        nc.vector.tensor_mul(w1e[:, 0:HH], w1sb[:], A0R[:])
        tmp1 = cp.tile([IN, HH], F32)
        nc.vector.tensor_mul(tmp1[:], w1sb[:], as1bR[:])
        for h in range(H):
            nc.vector.reduce_sum(w1e[:, HH + h:HH + h + 1],
                                 tmp1[:, h * HID:(h + 1) * HID],
                                 axis=mybir.AxisListType.X)
        nc.vector.tensor_mul(tmp1[:], w1sb[:], ad1bR[:])
        for h in range(H):
            nc.vector.reduce_sum(w1e[:, HH + H + h:HH + H + h + 1],
                                 tmp1[:, h * HID:(h + 1) * HID],
                                 axis=mybir.AxisListType.X)
        w1e16 = cp.tile([IN, HH + 2 * H], F16)
        nc.vector.tensor_copy(w1e16[:], w1e[:])

        # W2ext [HH, TW2] = [W2*A2 | w_as2 | w_ad2 | 0 0]
        w2e = cp.tile([HH, TW2], F32)
        nc.vector.memset(w2e[:], 0.0)
        nc.vector.tensor_mul(w2e[:, 0:HID], w2sb[:], A2R[:])
        tmp2 = cp.tile([HH, HID], F32)
        nc.vector.tensor_mul(tmp2[:], w2sb[:], as2bR[:])
        nc.vector.reduce_sum(w2e[:, HID:HID + 1], tmp2[:], axis=mybir.AxisListType.X)
        nc.vector.tensor_mul(tmp2[:], w2sb[:], ad2bR[:])
        nc.vector.reduce_sum(w2e[:, HID + 1:HID + 2], tmp2[:], axis=mybir.AxisListType.X)
        w2e16 = cp.tile([HH, TW2], F16)
        nc.vector.tensor_copy(w2e16[:], w2e[:])

        ones1 = cp.tile([128, 1], F16)
        nc.vector.memset(ones1[:], 1.0)
        identsb = cp.tile([128, 128], F16)
        nc.vector.tensor_scalar(identsb[:], iotaR[:], iotaPsb[:, 0:1], None,
                                op0=ALU.is_equal)
        wc16 = cp.tile([HID, OUT], F16)
        nc.vector.tensor_copy(wc16[:], wcsb[:])
        biasc = cp.tile([OUT, 1], F32)
        nc.vector.reduce_sum(biasc[:], wctsb[:], axis=mybir.AxisListType.X)
        nc.vector.tensor_sub(biasc[:], bctsb[:], biasc[:])

        # csum2 = ones^T @ W2ext (for the elu(+1) fold)
        csum2R = cp.tile([128, TW2], F32)
        with tc.tile_pool(name="cs_ps", bufs=1, space="PSUM") as cs_pool:
            cs_ps = cs_pool.tile([1, TW2], F32)
            nc.tensor.matmul(cs_ps[:], lhsT=ones1[:], rhs=w2e16[:], start=True, stop=True)
            csrow = cp.tile([1, TW2], F32)
            nc.vector.tensor_copy(csrow[:], cs_ps[:])
            dcs = nc.sync.dma_start(cs2d[:, :], csrow[:])
        d = nc.sync.dma_start(out=csum2R[:], in_=_bcast(cs2d[:, :], [[0, 128], [1, TW2]]))
        tile.add_dep_helper(d.ins, dcs.ins, sync=True)

        # ============ phase 1: table1 build + a_d mini-pass ============
        with tc.tile_pool(name="tb_x", bufs=2) as tbx, \
             tc.tile_pool(name="tb_st", bufs=4) as tbst, \
             tc.tile_pool(name="tb_ps", bufs=3, space="PSUM") as tbps:
            nchunk = math.ceil(NT / CT)
            for chi in range(nchunk):
                t0 = chi * CT
                t1 = min(NT, t0 + CT)
                xc = tbx.tile([IN, (t1 - t0) * 128], F16, tag="xc")
                nc.gpsimd.dma_start(out=xc[:], in_=xT[:, t0 * 128:t1 * 128])
                for t in range(t0, t1):
                    ps = tbps.tile([128, C1], F32, tag="ps")
                    nc.tensor.matmul(ps[:], lhsT=xc[:, (t - t0) * 128:(t - t0 + 1) * 128],
                                     rhs=w1e16[:, 0:C1], start=True, stop=True)
                    st = tbst.tile([128, C1], F16, tag="st")
                    nc.any.tensor_copy(st[:], ps[:])
                    nc.sync.dma_start(table1[t * 128:(t + 1) * 128, :], st[:])

            # a_d chunks for the core's own shard (per-core xT2 input)
            adsb16 = cp.tile([128, W * H], F16)
            nchunk2 = math.ceil(W / CT)
            for chi in range(nchunk2):
                t0 = chi * CT
                t1 = min(W, t0 + CT)
                xc = tbx.tile([IN, (t1 - t0) * 128], F16, tag="xc")
                nc.gpsimd.dma_start(out=xc[:], in_=xT2[:, t0 * 128:t1 * 128])
                for t in range(t0, t1):
                    ps2 = tbps.tile([128, H], F32, tag="ps2")
                    nc.tensor.matmul(ps2[:], lhsT=xc[:, (t - t0) * 128:(t - t0 + 1) * 128],
                                     rhs=w1e16[:, C1:C1 + H], start=True, stop=True)
                    nc.vector.tensor_copy(adsb16[:, t * H:(t + 1) * H], ps2[:])

        adsb2_16 = cp.tile([128, W], F16)

        t2_dmas = []

        # ============ edge phase (shared for both layers) ============
        def edge_layer(layer, poolps=None):
            if layer == 1:
                tbl, tblW, nh, nx = table1, C1, H, HH
                src_t = srcP
            else:
                tbl, tblW, nh, nx = table2, TW2, 1, HID
                src_t = src2P
            ncol = nx + nh

            pw = {}
            sfx = "a" if layer == 1 else "b"
            with tc.tile_pool(name="e_idx" + sfx, bufs=2) as pidx, \
                 tc.tile_pool(name="e_dlp" + sfx, bufs=2) as pdlp, \
                 tc.tile_pool(name="e_dlf" + sfx, bufs=2) as pdlf, \
                 tc.tile_pool(name="e_x" + sfx, bufs=3) as px, \
                 tc.tile_pool(name="e_s" + sfx, bufs=2) as ps_, \
                 tc.tile_pool(name="e_st" + sfx, bufs=2) as pst, \
                 tc.tile_pool(name="e_e" + sfx, bufs=2) as pe_, \
                 tc.tile_pool(name="e_ade" + sfx, bufs=2, space="PSUM") as pade, \
                 tc.tile_pool(name="e_pw" + sfx, bufs=3, space="PSUM") as ppw, \
                 tc.tile_pool(name="ev_f" + sfx, bufs=2) as pef, \
                 tc.tile_pool(name="ev_ps" + sfx, bufs=1, space="PSUM") as pev:

                nchunks = math.ceil(NB / KB)
                for ci in range(nchunks):
                    b0 = ci * KB
                    k = min(NB, b0 + KB) - b0
                    sidx = pidx.tile([128, k], I32, tag="i")
                    nc.sync.dma_start(sidx[:], src_t[:, b0:b0 + k])
                    dlp = pdlp.tile([128, k], F16, tag="d")
                    nc.sync.dma_start(dlp[:], dlP[:, b0:b0 + k])
                    dlfR = pdlf.tile([128, k * 128], F16, tag="f")
                    nc.sync.dma_start(
                        out=dlfR[:],
                        in_=_bcast(dlF[0:1, b0 * 128:(b0 + k) * 128],
                                   [[0, 128], [1, k * 128]]))

                    X = px.tile([128, k, tblW], F16, tag="x")
                    for j in range(k):
                        gi = nc.gpsimd.indirect_dma_start(
                            out=X[:, j, :], out_offset=None, in_=tbl[:, :],
                            in_offset=bass.IndirectOffsetOnAxis(
                                ap=sidx[:, j:j + 1], axis=0))
                        if layer == 2 and t2_dmas:
                            tile.add_dep_helper(gi.ins, t2_dmas[0].ins, sync=True)

                    S = ps_.tile([128, k * 128], F16, tag="s")
                    nc.vector.tensor_tensor(
                        S[:].rearrange("p (k d) -> p k d", k=k),
                        _bcast(dlp[:], [dlp[:].ap[0], [1, k], [0, 128]]),
                        _bcast(iotaR[:], [iotaR[:].ap[0], [0, k], [1, 128]]),
                        op=ALU.is_equal)
                    ST = pst.tile([128, k * 128], F16, tag="t")
                    nc.vector.tensor_scalar(
                        ST[:], dlfR[:], iotaPsb[:, 0:1], None, op0=ALU.is_equal)

                    adE = pade.tile([128, k * nh], F32, tag="a")
                    for j in range(k):
                        w = int(blk_win[b0 + j])
                        if layer == 1:
                            rhs = adsb16[:, w * H:(w + 1) * H]
                        else:
                            rhs = adsb2_16[:, w:w + 1]
                        nc.tensor.matmul(adE[:, j * nh:(j + 1) * nh],
                                         lhsT=ST[:, j * 128:(j + 1) * 128],
                                         rhs=rhs, start=True, stop=True,
                                         skip_group_check=True)

                    e = pe_.tile([128, k * nh], F32, tag="e")
                    nc.vector.tensor_tensor(
                        e[:].rearrange("p (k h) -> p k h", k=k),
                        X[:, :, nx:nx + nh],
                        adE[:].rearrange("p (k h) -> p k h", k=k),
                        op=ALU.add)
                    nc.vector.scalar_tensor_tensor(
                        e[:], e[:], SLOPE, e[:], op0=ALU.mult, op1=ALU.max)
                    nc.scalar.activation(X[:, :, nx:nx + nh],
                                         e[:].rearrange("p (k h) -> p k h", k=k),
                                         AF.Exp)
                    xa = X[:].ap
                    exb = _bcast(X[:, :, nx:nx + nh],
                                 [xa[0], [tblW, k], [1, nh], [0, HID]])
                    nc.vector.tensor_tensor(
                        X[:, :, 0:nx].rearrange("p k (h c) -> p k h c", h=nh),
                        X[:, :, 0:nx].rearrange("p k (h c) -> p k h c", h=nh),
                        exb, op=ALU.mult)

                    for j in range(k):
                        b = b0 + j
                        w = int(blk_win[b])
                        if b == wb0[w]:
                            pw[w] = ppw.tile([128, ncol], F32, name=f"pw{w}", tag="w")
                        nc.tensor.matmul(pw[w][:], lhsT=S[:, j * 128:(j + 1) * 128],
                                         rhs=X[:, j, 0:ncol],
                                         start=(b == wb0[w]), stop=(b == wbl[w]),
                                         skip_group_check=True)

                    # ---- evacuate windows finishing in this chunk ----
                    for w in range(W):
                        if not (b0 <= wbl[w] < b0 + k):
                            continue
                        rden = pef.tile([128, nh], F32, tag="rd")
                        nc.vector.tensor_scalar_max(rden[:], pw[w][:, nx:nx + nh], 1e-6)
                        r = pef.tile([128, nh], F32, tag="r")
                        nc.vector.reciprocal(r[:], rden[:])
                        u = pef.tile([128, nx], F32, tag="u")
                        ra = r[:].ap
                        nc.vector.tensor_tensor(
                            u[:].rearrange("p (h c) -> p h c", h=nh),
                            pw[w][:, 0:nx].rearrange("p (h c) -> p h c", h=nh),
                            _bcast(r[:], [ra[0], [1, nh], [0, HID]]),
                            op=ALU.mult)
                        BvR = B0R if layer == 1 else B2R
                        nc.vector.tensor_add(u[:], u[:], BvR[:, 0:nx])
                        ru = pef.tile([128, nx], F32, tag="ru")
                        nc.scalar.activation(ru[:], u[:], AF.Relu)
                        pex = pef.tile([128, nx], F32, tag="pe")
                        nc.scalar.activation(pex[:], u[:], AF.Relu, scale=-1.0)
                        nc.scalar.activation(pex[:], pex[:], AF.Exp, scale=-1.0)

                        if layer == 1:
                            h1c = pef.tile([128, nx], F16, tag="h1")
                            nc.vector.tensor_add(h1c[:], ru[:], pex[:])
                            tp = pev.tile([128, HH], F16, tag="tp")
                            nc.tensor.transpose(tp[:], h1c[:], identsb[:])
                            h1T = pef.tile([128, HH], F16, tag="ht")
                            nc.any.tensor_copy(h1T[:], tp[:])
                            x2 = pev.tile([128, TW2], F32, tag="x2")
                            nc.tensor.matmul(x2[:], lhsT=h1T[:], rhs=w2e16[:],
                                             start=True, stop=True,
                                             skip_group_check=True)
                            sh2 = pef.tile([128, TW2], F16, tag="s2")
                            nc.vector.tensor_sub(sh2[:], x2[:], csum2R[:])
                            nc.vector.tensor_copy(adsb2_16[:, w:w + 1],
                                                  sh2[:, HID + 1:HID + 2])
                            dd = nc.sync.dma_start(t2in[w * 128:(w + 1) * 128, :],
                                                   sh2[:])
                            t2_dmas.append(dd)
                        else:
                            h2e = pef.tile([128, HID + 1], F16, tag="h2")
                            nc.vector.tensor_add(h2e[:, 0:HID], ru[:], pex[:])
                            nc.vector.memset(h2e[:, HID:HID + 1], 1.0)
                            Gt = pef.tile([128, G], F16, tag="g")
                            gga = glsb[:].ap
                            nc.vector.tensor_tensor(
                                Gt[:],
                                _bcast(glsb[:, w:w + 1], [gga[0], [0, G]]),
                                iotaGR[:], op=ALU.is_equal)
                            nc.tensor.matmul(poolps[:], lhsT=h2e[:], rhs=Gt[:],
                                             start=(w == 0), stop=(w == W - 1),
                                             skip_group_check=True)
            return

        edge_layer(1)

        # ============ phase 4: AllGather table2 ============
        cc1 = nc.gpsimd.collective_compute(
            "AllGather", ALU.bypass, replica_groups=rg,
            ins=[t2in[:, :]], outs=[table2[:, :]])
        for dd in t2_dmas:
            tile.add_dep_helper(cc1.ins, dd.ins, sync=True)
        t2_dmas = [cc1]

        # ============ phase 5: L2 edge loop + pooling ============
        with tc.tile_pool(name="poolpsp", bufs=1, space="PSUM") as ppool:
            poolps = ppool.tile([HID + 1, G], F32)
            edge_layer(2, poolps=poolps)

            # ============ phase 6: finale ============
            poolsb = cp.tile([HID + 1, G], F32)
            nc.any.tensor_copy(poolsb[:], poolps[:])
        dpi = nc.sync.dma_start(poolin[:, :], poolsb[:])
        cc2 = nc.gpsimd.collective_compute(
            "AllReduce", ALU.add, replica_groups=rg,
            ins=[poolin[:, :]], outs=[poolout[:, :]])
        tile.add_dep_helper(cc2.ins, dpi.ins, sync=True)
        psb = cp.tile([HID + 1, G], F32)
        d2 = nc.sync.dma_start(psb[:], poolout[:, :])
        tile.add_dep_helper(d2.ins, cc2.ins, sync=True)
        cntt = cp.tile([1, G], F32)
        nc.vector.tensor_scalar_max(cntt[:], psb[HID:HID + 1, :], 1.0)
        rec = cp.tile([1, G], F32)
        nc.vector.reciprocal(rec[:], cntt[:])
        nmk = cp.tile([1, G], F32)
        nc.vector.tensor_scalar(nmk[:], psb[HID:HID + 1, :], 0.5, None, op0=ALU.is_lt)
        drc = nc.sync.dma_start(recd[:, :], rec[:])
        dnm = nc.sync.dma_start(nmd[:, :], nmk[:])
        recR = cp.tile([HID, G], F32)
        d3 = nc.sync.dma_start(out=recR[:], in_=_bcast(recd[:, :], [[0, HID], [1, G]]))
        tile.add_dep_helper(d3.ins, drc.ins, sync=True)
        nmR = cp.tile([HID, G], F32)
        d4 = nc.sync.dma_start(out=nmR[:], in_=_bcast(nmd[:, :], [[0, HID], [1, G]]))
        tile.add_dep_helper(d4.ins, dnm.ins, sync=True)
        mean0 = cp.tile([HID, G], F32)
        nc.vector.tensor_tensor(mean0[:], psb[0:HID, :], recR[:], op=ALU.mult)
        meanc = cp.tile([HID, G], F16)
        nc.vector.tensor_tensor(meanc[:], mean0[:], nmR[:], op=ALU.add)
        with tc.tile_pool(name="fin_ps", bufs=1, space="PSUM") as pfin:
            finps = pfin.tile([OUT, G], F32)
            nc.tensor.matmul(finps[:], lhsT=wc16[:], rhs=meanc[:], start=True, stop=True)
            osb = cp.tile([OUT, G], F32)
            nc.vector.tensor_scalar_add(osb[:], finps[:], biasc[:, 0:1])
        nc.sync.dma_start(outT[:, :], osb[:])

    nc.finalize()
    return nc


# --------------------------------------------------------------------------
# host entry point
# --------------------------------------------------------------------------
def make_in_maps(inputs, P, n_cores):
    N = inputs["x"].shape[0]
    NPAD, NP, SH = P["NPAD"], P["NP"], P["SH"]
    x = np.asarray(inputs["x"], np.float32)
    xT = np.zeros((IN, NPAD), np.float32)
    xT[:, :N] = x.T
    Wc = np.asarray(inputs["Wc"], np.float32)
    common = {
        "xT": xT,
        "W1": np.asarray(inputs["W1"], np.float32),
        "as1": np.asarray(inputs["att_src1"], np.float32),
        "ad1": np.asarray(inputs["att_dst1"], np.float32),
        "g1": np.asarray(inputs["gamma1"], np.float32).reshape(1, -1),
        "be1": np.asarray(inputs["beta1"], np.float32).reshape(1, -1),
        "rm1": np.asarray(inputs["rm1"], np.float32).reshape(1, -1),
        "rv1": np.asarray(inputs["rv1"], np.float32).reshape(1, -1),
        "bb1": np.asarray(inputs["b1"], np.float32).reshape(1, -1),
        "W2": np.asarray(inputs["W2"], np.float32),
        "as2": np.asarray(inputs["att_src2"], np.float32),
        "ad2": np.asarray(inputs["att_dst2"], np.float32),
        "g2": np.asarray(inputs["gamma2"], np.float32).reshape(1, -1),
        "be2": np.asarray(inputs["beta2"], np.float32).reshape(1, -1),
        "rm2": np.asarray(inputs["rm2"], np.float32).reshape(1, -1),
        "rv2": np.asarray(inputs["rv2"], np.float32).reshape(1, -1),
        "bb2": np.asarray(inputs["b2"], np.float32).reshape(1, -1),
        "Wc": Wc,
        "WcT": np.ascontiguousarray(Wc.T),
        "bcT": np.asarray(inputs["bc"], np.float32).reshape(-1, 1),
        "iota128": np.arange(128, dtype=np.float16).reshape(1, 128),
        "iotaG": np.arange(G, dtype=np.float16).reshape(1, G),
        "iotaPf": np.arange(128, dtype=np.float32).reshape(128, 1),
    }
    in_maps = []
    for c in range(n_cores):
        m = dict(common)
        xT2 = np.zeros((IN, NP), np.float32)
        hi = min(N, c * SH + NP)
        xT2[:, :hi - c * SH] = x[c * SH:hi].T
        m["xT2"] = xT2
        for nm in ["srcP", "src2P", "dlP", "dlF", "glP"]:
            m[nm] = P[nm][c]
        in_maps.append(m)
    return in_maps


def kernel(**inputs):
    ei = np.asarray(inputs["edge_index"]).astype(np.int64)
    batch = np.asarray(inputs["batch"]).astype(np.int64)
    N = inputs["x"].shape[0]
    P = _prep(ei, batch, N, NC)
    meta = dict(P)
    meta["n_cores"] = NC
    nc = _build(meta)
    in_maps = make_in_maps(inputs, P, NC)
    import time as _time
    t0 = _time.monotonic()
    res = run_bass_kernel_spmd(nc, in_maps, core_ids=list(range(NC)))
    globals()["LAST_RUN_S"] = _time.monotonic() - t0
    outT = np.asarray(res.results[0]["outT"])
    return np.ascontiguousarray(outT.T).astype(np.float32)


# revision 16
# speedup vs baseline: 1.9143x; 1.9143x over previous
"""Sharded GAT graph classifier for Trainium2 (Bass/Tile, 8 NeuronCores).

Strategy (1D dst-partitioning, per the sharding hint):
 - nodes (message destinations) sharded across 8 cores; each core owns all
   edges whose dst falls in its shard (self-loops included as edges),
   sorted/grouped into 128-dst windows
 - per-core: layer-1 node table (BN-folded projected features + src attention
   logits, fp16) rebuilt locally, then the edge phase gathers source rows with
   one indirect DMA per 128-edge block and aggregates messages with one-hot
   segment-sum matmuls into PSUM windows; dst logits are expanded per edge
   with a one-hot-transpose matmul against the window's a_d chunk
 - layer-2 table shards exchanged with one AllGather; pooling partials
   combined with one AllReduce; classifier replicated
 - softmax uses exp without max-subtraction (logits bounded ~|4.3|, fp32 exp)
 - BN folded into table scales; ELU carried as elu(x)+1 with the -1 folded
   into downstream bias columns
"""
import math
import numpy as np
import ml_dtypes  # noqa: F401
from contextlib import ExitStack

import concourse.bass as bass
import concourse.tile as tile
from concourse import bacc, mybir
from concourse.bass_utils import run_bass_kernel_spmd

F16 = mybir.dt.float16
F32 = mybir.dt.float32
I32 = mybir.dt.int32
AF = mybir.ActivationFunctionType
ALU = mybir.AluOpType

BN_EPS = 1e-5
SLOPE = 0.2

G = 512
IN = 64
H = 4
HID = 32
OUT = 2
HH = H * HID            # 128
C1 = HH + H             # 132 f16 units per table1 row: [xs' x128 | a_s x4]
TW2 = HID + 4           # 36 f16 units per table2 row: [xs2' x32 | a_s2 | a_d2 | 0 0]
NC = 8

KB = 32                 # edge blocks per chunk
CT = 16                 # node tiles per cast chunk in table build


# --------------------------------------------------------------------------
# host-side index/layout preprocessing (no FP math on data)
# --------------------------------------------------------------------------
def _prep(edge_index, batch, N, n_cores):
    SH = N // n_cores
    W = (SH + 127) // 128
    NP = W * 128
    NT = (N + 127) // 128
    NPAD = NT * 128
    assert n_cores * SH == N

    ar = np.arange(N, dtype=np.int64)
    src = np.concatenate([edge_index[0].astype(np.int64), ar])
    dst = np.concatenate([edge_index[1].astype(np.int64), ar])
    E = src.shape[0]

    core = dst // SH
    loc = dst % SH
    win = loc // 128
    dstloc = loc % 128

    cnt = np.zeros((n_cores, W), dtype=np.int64)
    np.add.at(cnt, (core, win), 1)
    blocks_w = np.maximum((cnt.max(axis=0) + 127) // 128, 1)
    NB = int(blocks_w.sum())
    L = NB * 128

    order = np.lexsort((win, core))
    s_src, s_core, s_win = src[order], core[order], win[order]
    s_dstloc = dstloc[order]

    wstart = np.zeros(W, dtype=np.int64)
    wstart[1:] = np.cumsum(blocks_w * 128)[:-1]

    src_idx = np.zeros((n_cores, L), dtype=np.int64)
    dl = np.full((n_cores, L), 300, dtype=np.int64)
    grp = s_core * W + s_win
    grp_start = np.zeros(n_cores * W + 1, dtype=np.int64)
    np.add.at(grp_start, grp + 1, 1)
    grp_start = np.cumsum(grp_start)
    pos = np.arange(E) - grp_start[grp]
    slot = wstart[s_win] + pos
    src_idx[s_core, slot] = s_src
    dl[s_core, slot] = s_dstloc

    blk_win = np.repeat(np.arange(W), blocks_w)

    def pmaj(a, dt):
        return np.ascontiguousarray(
            a.reshape(n_cores, NB, 128).transpose(0, 2, 1)).astype(dt)

    srcP = pmaj(src_idx, np.int32)
    dlP = pmaj(dl, np.float16)
    dlF = np.ascontiguousarray(dl.astype(np.float16)).reshape(n_cores, 1, L)
    src2_idx = (src_idx // SH) * NP + (src_idx % SH)
    src2P = pmaj(src2_idx, np.int32)

    gl = np.full((n_cores, NP), 60000, dtype=np.int64)
    for c in range(n_cores):
        gl[c, :SH] = batch[c * SH:(c + 1) * SH]
    glP = np.ascontiguousarray(
        gl.reshape(n_cores, W, 128).transpose(0, 2, 1)).astype(np.float16)

    TS = (NT + n_cores - 1) // n_cores
    return dict(SH=SH, W=W, NP=NP, NT=NT, NPAD=NPAD, NB=NB, TS=TS,
                blocks_w=blocks_w, blk_win=blk_win,
                srcP=srcP, dlP=dlP, dlF=dlF, src2P=src2P, glP=glP)


def _bcast(ap, pattern):
    """AP on ap's tensor with an explicit [step, count] list."""
    return bass.AP(ap.tensor, ap.offset, [list(x) for x in pattern])


# --------------------------------------------------------------------------
# device program
# --------------------------------------------------------------------------
def _build(meta):
    W, NB, NPAD, NP = meta["W"], meta["NB"], meta["NPAD"], meta["NP"]
    NT = meta["NT"]
    TS = meta["TS"]
    blk_win = meta["blk_win"]
    n_cores = meta["n_cores"]

    wb0 = np.zeros(W, dtype=np.int64)
    wbl = np.zeros(W, dtype=np.int64)
    for w in range(W):
        idxs = np.where(blk_win == w)[0]
        wb0[w], wbl[w] = idxs[0], idxs[-1]

    nc = bacc.Bacc("TRN2", num_devices=n_cores)

    def dinp(name, shape, dt):
        return nc.dram_tensor(name, shape, dt, kind="ExternalInput")

    xTs = dinp("xTs", [IN, TS * 128], F32)
    xT2 = dinp("xT2", [IN, NP], F32)
    W1 = dinp("W1", [IN, HH], F32)
    as1 = dinp("as1", [H, HID], F32)
    ad1 = dinp("ad1", [H, HID], F32)
    g1 = dinp("g1", [1, HH], F32); be1 = dinp("be1", [1, HH], F32)
    rm1 = dinp("rm1", [1, HH], F32); rv1 = dinp("rv1", [1, HH], F32)
    bb1 = dinp("bb1", [1, HH], F32)
    W2 = dinp("W2", [HH, HID], F32)
    as2 = dinp("as2", [1, HID], F32)
    ad2 = dinp("ad2", [1, HID], F32)
    g2 = dinp("g2", [1, HID], F32); be2 = dinp("be2", [1, HID], F32)
    rm2 = dinp("rm2", [1, HID], F32); rv2 = dinp("rv2", [1, HID], F32)
    bb2 = dinp("bb2", [1, HID], F32)
    Wc = dinp("Wc", [HID, OUT], F32)
    WcT = dinp("WcT", [OUT, HID], F32)
    bcT = dinp("bcT", [OUT, 1], F32)
    srcP = dinp("srcP", [128, NB], I32)
    src2P = dinp("src2P", [128, NB], I32)
    dlP = dinp("dlP", [128, NB], F16)
    dlF = dinp("dlF", [1, NB * 128], F16)
    glP = dinp("glP", [128, W], F16)
    iota128 = dinp("iota128", [1, 128], F16)
    iotaG = dinp("iotaG", [1, G], F16)
    iotaPf = dinp("iotaPf", [128, 1], F32)

    t1in = nc.dram_tensor("t1in", [TS * 128, C1], F16)
    table1 = nc.dram_tensor("table1", [n_cores * TS * 128, C1], F16,
                            addr_space="Shared")
    t2in = nc.dram_tensor("t2in", [NP, TW2], F16)
    table2 = nc.dram_tensor("table2", [n_cores * NP, TW2], F16, addr_space="Shared")
    poolin = nc.dram_tensor("poolin", [HID + 1, G], F32)
    poolout = nc.dram_tensor("poolout", [HID + 1, G], F32, addr_space="Shared")
    b0d = nc.dram_tensor("b0d", [1, HH], F32)
    b2d = nc.dram_tensor("b2d", [1, HID], F32)
    a0d = nc.dram_tensor("a0d", [1, HH], F32)
    a2d = nc.dram_tensor("a2d", [1, HID], F32)
    cs2d = nc.dram_tensor("cs2d", [1, TW2], F32)
    recd = nc.dram_tensor("recd", [1, G], F32)
    nmd = nc.dram_tensor("nmd", [1, G], F32)
    outT = nc.dram_tensor("outT", [OUT, G], F32, kind="ExternalOutput")

    rg = [list(range(n_cores))]

    with ExitStack() as ctx:
        tc = ctx.enter_context(tile.TileContext(nc))
        cp = ctx.enter_context(tc.tile_pool(name="consts", bufs=1))

        # ============ phase 0: constants ============
        w1sb = cp.tile([IN, HH], F32)
        nc.sync.dma_start(w1sb[:], W1[:, :])
        w2sb = cp.tile([HH, HID], F32)
        nc.sync.dma_start(w2sb[:], W2[:, :])
        rows1 = {}
        for nm, t in [("g1", g1), ("be1", be1), ("rm1", rm1), ("rv1", rv1), ("bb1", bb1)]:
            rows1[nm] = cp.tile([1, HH], F32, name="r" + nm, tag="r" + nm)
            nc.sync.dma_start(rows1[nm][:], t[:, :])
        rows2 = {}
        for nm, t in [("g2", g2), ("be2", be2), ("rm2", rm2), ("rv2", rv2), ("bb2", bb2)]:
            rows2[nm] = cp.tile([1, HID], F32, name="r" + nm, tag="r" + nm)
            nc.sync.dma_start(rows2[nm][:], t[:, :])
        wcsb = cp.tile([HID, OUT], F32); nc.sync.dma_start(wcsb[:], Wc[:, :])
        wctsb = cp.tile([OUT, HID], F32); nc.sync.dma_start(wctsb[:], WcT[:, :])
        bctsb = cp.tile([OUT, 1], F32); nc.sync.dma_start(bctsb[:], bcT[:, :])
        glsb = cp.tile([128, W], F16); nc.sync.dma_start(glsb[:], glP[:, :])
        iotaPsb = cp.tile([128, 1], F32)
        nc.sync.dma_start(iotaPsb[:], iotaPf[:, :])
        # partition-replicated constants (HWDGE broadcast reads)
        iotaR = cp.tile([128, 128], F16)
        nc.sync.dma_start(out=iotaR[:], in_=_bcast(iota128[:, :], [[0, 128], [1, 128]]))
        iotaGR = cp.tile([128, G], F16)
        nc.sync.dma_start(out=iotaGR[:], in_=_bcast(iotaG[:, :], [[0, 128], [1, G]]))
        as1bR = cp.tile([IN, HH], F32)
        nc.sync.dma_start(out=as1bR[:], in_=_bcast(as1[:, :], [[0, IN], [1, HH]]))
        ad1bR = cp.tile([IN, HH], F32)
        nc.sync.dma_start(out=ad1bR[:], in_=_bcast(ad1[:, :], [[0, IN], [1, HH]]))
        as2bR = cp.tile([HH, HID], F32)
        nc.sync.dma_start(out=as2bR[:], in_=_bcast(as2[:, :], [[0, HH], [1, HID]]))
        ad2bR = cp.tile([HH, HID], F32)
        nc.sync.dma_start(out=ad2bR[:], in_=_bcast(ad2[:, :], [[0, HH], [1, HID]]))

        def bn_fold(rows, width, ad_, bd_):
            sd = cp.tile([1, width], F32, name="sd%d" % width, tag="sd%d" % width)
            nc.vector.tensor_scalar_add(sd[:], rows["rv"][:], BN_EPS)
            nc.scalar.activation(sd[:], sd[:], AF.Sqrt)
            rs = cp.tile([1, width], F32, name="rs%d" % width, tag="rs%d" % width)
            nc.vector.reciprocal(rs[:], sd[:])
            A = cp.tile([1, width], F32, name="A%d" % width, tag="A%d" % width)
            nc.vector.tensor_mul(A[:], rs[:], rows["g"][:])
            B = cp.tile([1, width], F32, name="B%d" % width, tag="B%d" % width)
            nc.vector.tensor_sub(B[:], rows["bb"][:], rows["rm"][:])
            nc.vector.tensor_mul(B[:], B[:], A[:])
            nc.vector.tensor_add(B[:], B[:], rows["be"][:])
            da = nc.sync.dma_start(ad_[:, :], A[:])
            db = nc.sync.dma_start(bd_[:, :], B[:])
            return da, db

        da0, db0 = bn_fold({"rv": rows1["rv1"], "g": rows1["g1"], "bb": rows1["bb1"],
                            "rm": rows1["rm1"], "be": rows1["be1"]}, HH, a0d, b0d)
        da2, db2 = bn_fold({"rv": rows2["rv2"], "g": rows2["g2"], "bb": rows2["bb2"],
                            "rm": rows2["rm2"], "be": rows2["be2"]}, HID, a2d, b2d)
        A0R = cp.tile([IN, HH], F32)
        d = nc.sync.dma_start(out=A0R[:], in_=_bcast(a0d[:, :], [[0, IN], [1, HH]]))
        tile.add_dep_helper(d.ins, da0.ins, sync=True)
        B0R = cp.tile([128, HH], F32)
        d = nc.sync.dma_start(out=B0R[:], in_=_bcast(b0d[:, :], [[0, 128], [1, HH]]))
        tile.add_dep_helper(d.ins, db0.ins, sync=True)
        A2R = cp.tile([HH, HID], F32)
        d = nc.sync.dma_start(out=A2R[:], in_=_bcast(a2d[:, :], [[0, HH], [1, HID]]))
        tile.add_dep_helper(d.ins, da2.ins, sync=True)
        B2R = cp.tile([128, HID], F32)
        d = nc.sync.dma_start(out=B2R[:], in_=_bcast(b2d[:, :], [[0, 128], [1, HID]]))
        tile.add_dep_helper(d.ins, db2.ins, sync=True)

        # W1ext [IN, 136] = [W1*A0 | w_as1 | w_ad1]
        w1e = cp.tile([IN, HH + 2 * H], F32)
        nc.vector.tensor_mul(w1e[:, 0:HH], w1sb[:], A0R[:])
        tmp1 = cp.tile([IN, HH], F32)
        nc.vector.tensor_mul(tmp1[:], w1sb[:], as1bR[:])
        for h in range(H):
            nc.vector.reduce_sum(w1e[:, HH + h:HH + h + 1],
                                 tmp1[:, h * HID:(h + 1) * HID],
                                 axis=mybir.AxisListType.X)
        nc.vector.tensor_mul(tmp1[:], w1sb[:], ad1bR[:])
        for h in range(H):
            nc.vector.reduce_sum(w1e[:, HH + H + h:HH + H + h + 1],
                                 tmp1[:, h * HID:(h + 1) * HID],
                                 axis=mybir.AxisListType.X)
        w1e16 = cp.tile([IN, HH + 2 * H], F16)
        nc.vector.tensor_copy(w1e16[:], w1e[:])

        # W2ext [HH, TW2] = [W2*A2 | w_as2 | w_ad2 | 0 0]
        w2e = cp.tile([HH, TW2], F32)
        nc.vector.memset(w2e[:], 0.0)
        nc.vector.tensor_mul(w2e[:, 0:HID], w2sb[:], A2R[:])
        tmp2 = cp.tile([HH, HID], F32)
        nc.vector.tensor_mul(tmp2[:], w2sb[:], as2bR[:])
        nc.vector.reduce_sum(w2e[:, HID:HID + 1], tmp2[:], axis=mybir.AxisListType.X)
        nc.vector.tensor_mul(tmp2[:], w2sb[:], ad2bR[:])
        nc.vector.reduce_sum(w2e[:, HID + 1:HID + 2], tmp2[:], axis=mybir.AxisListType.X)
        w2e16 = cp.tile([HH, TW2], F16)
        nc.vector.tensor_copy(w2e16[:], w2e[:])

        ones1 = cp.tile([128, 1], F16)
        nc.vector.memset(ones1[:], 1.0)
        identsb = cp.tile([128, 128], F16)
        nc.vector.tensor_scalar(identsb[:], iotaR[:], iotaPsb[:, 0:1], None,
                                op0=ALU.is_equal)
        wc16 = cp.tile([HID, OUT], F16)
        nc.vector.tensor_copy(wc16[:], wcsb[:])
        biasc = cp.tile([OUT, 1], F32)
        nc.vector.reduce_sum(biasc[:], wctsb[:], axis=mybir.AxisListType.X)
        nc.vector.tensor_sub(biasc[:], bctsb[:], biasc[:])

        # csum2 = ones^T @ W2ext (for the elu(+1) fold)
        csum2R = cp.tile([128, TW2], F32)
        with tc.tile_pool(name="cs_ps", bufs=1, space="PSUM") as cs_pool:
            cs_ps = cs_pool.tile([1, TW2], F32)
            nc.tensor.matmul(cs_ps[:], lhsT=ones1[:], rhs=w2e16[:], start=True, stop=True)
            csrow = cp.tile([1, TW2], F32)
            nc.vector.tensor_copy(csrow[:], cs_ps[:])
            dcs = nc.sync.dma_start(cs2d[:, :], csrow[:])
        d = nc.sync.dma_start(out=csum2R[:], in_=_bcast(cs2d[:, :], [[0, 128], [1, TW2]]))
        tile.add_dep_helper(d.ins, dcs.ins, sync=True)

        # ============ phase 1: sharded table1 build + AllGather ============
        t1_dmas = []
        with tc.tile_pool(name="tb_x", bufs=2) as tbx, \
             tc.tile_pool(name="tb_st", bufs=4) as tbst, \
             tc.tile_pool(name="tb_ps", bufs=3, space="PSUM") as tbps:
            nchunk = math.ceil(TS / CT)
            for chi in range(nchunk):
                t0 = chi * CT
                t1 = min(TS, t0 + CT)
                xc = tbx.tile([IN, (t1 - t0) * 128], F16, tag="xc")
                nc.gpsimd.dma_start(out=xc[:], in_=xTs[:, t0 * 128:t1 * 128])
                for t in range(t0, t1):
                    ps = tbps.tile([128, C1], F32, tag="ps")
                    nc.tensor.matmul(ps[:], lhsT=xc[:, (t - t0) * 128:(t - t0 + 1) * 128],
                                     rhs=w1e16[:, 0:C1], start=True, stop=True)
                    st = tbst.tile([128, C1], F16, tag="st")
                    nc.any.tensor_copy(st[:], ps[:])
                    dd = nc.sync.dma_start(t1in[t * 128:(t + 1) * 128, :], st[:])
                    t1_dmas.append(dd)

            # a_d chunks for the core's own shard (per-core xT2 input)
            adsb16 = cp.tile([128, W * H], F16)
            nchunk2 = math.ceil(W / CT)
            for chi in range(nchunk2):
                t0 = chi * CT
                t1 = min(W, t0 + CT)
                xc = tbx.tile([IN, (t1 - t0) * 128], F16, tag="xc")
                nc.gpsimd.dma_start(out=xc[:], in_=xT2[:, t0 * 128:t1 * 128])
                for t in range(t0, t1):
                    ps2 = tbps.tile([128, H], F32, tag="ps2")
                    nc.tensor.matmul(ps2[:], lhsT=xc[:, (t - t0) * 128:(t - t0 + 1) * 128],
                                     rhs=w1e16[:, C1:C1 + H], start=True, stop=True)
                    nc.vector.tensor_copy(adsb16[:, t * H:(t + 1) * H], ps2[:])

        cc0 = nc.gpsimd.collective_compute(
            "AllGather", ALU.bypass, replica_groups=rg,
            ins=[t1in[:, :]], outs=[table1[:, :]])
        for dd in t1_dmas:
            tile.add_dep_helper(cc0.ins, dd.ins, sync=True)

        adsb2_16 = cp.tile([128, W], F16)

        t2_dmas = []

        # ============ edge phase (shared for both layers) ============
        def edge_layer(layer, poolps=None):
            if layer == 1:
                tbl, tblW, nh, nx = table1, C1, H, HH
                src_t = srcP
            else:
                tbl, tblW, nh, nx = table2, TW2, 1, HID
                src_t = src2P
            ncol = nx + nh

            pw = {}
            sfx = "a" if layer == 1 else "b"
            with tc.tile_pool(name="e_idx" + sfx, bufs=2) as pidx, \
                 tc.tile_pool(name="e_dlp" + sfx, bufs=2) as pdlp, \
                 tc.tile_pool(name="e_dlf" + sfx, bufs=2) as pdlf, \
                 tc.tile_pool(name="e_x" + sfx, bufs=3) as px, \
                 tc.tile_pool(name="e_s" + sfx, bufs=2) as ps_, \
                 tc.tile_pool(name="e_st" + sfx, bufs=2) as pst, \
                 tc.tile_pool(name="e_e" + sfx, bufs=2) as pe_, \
                 tc.tile_pool(name="e_ade" + sfx, bufs=2, space="PSUM") as pade, \
                 tc.tile_pool(name="e_pw" + sfx, bufs=3, space="PSUM") as ppw, \
                 tc.tile_pool(name="ev_f" + sfx, bufs=2) as pef, \
                 tc.tile_pool(name="ev_ps" + sfx, bufs=1, space="PSUM") as pev:

                nchunks = math.ceil(NB / KB)
                for ci in range(nchunks):
                    b0 = ci * KB
                    k = min(NB, b0 + KB) - b0
                    sidx = pidx.tile([128, k], I32, tag="i")
                    nc.sync.dma_start(sidx[:], src_t[:, b0:b0 + k])
                    dlp = pdlp.tile([128, k], F16, tag="d")
                    nc.sync.dma_start(dlp[:], dlP[:, b0:b0 + k])
                    dlfR = pdlf.tile([128, k * 128], F16, tag="f")
                    nc.sync.dma_start(
                        out=dlfR[:],
                        in_=_bcast(dlF[0:1, b0 * 128:(b0 + k) * 128],
                                   [[0, 128], [1, k * 128]]))

                    X = px.tile([128, k, tblW], F16, tag="x")
                    for j in range(k):
                        gi = nc.gpsimd.indirect_dma_start(
                            out=X[:, j, :], out_offset=None, in_=tbl[:, :],
                            in_offset=bass.IndirectOffsetOnAxis(
                                ap=sidx[:, j:j + 1], axis=0))
                        if layer == 1:
                            tile.add_dep_helper(gi.ins, cc0.ins, sync=True)
                        elif t2_dmas:
                            tile.add_dep_helper(gi.ins, t2_dmas[0].ins, sync=True)

                    S = ps_.tile([128, k * 128], F16, tag="s")
                    nc.vector.tensor_tensor(
                        S[:].rearrange("p (k d) -> p k d", k=k),
                        _bcast(dlp[:], [dlp[:].ap[0], [1, k], [0, 128]]),
                        _bcast(iotaR[:], [iotaR[:].ap[0], [0, k], [1, 128]]),
                        op=ALU.is_equal)
                    ST = pst.tile([128, k * 128], F16, tag="t")
                    nc.vector.tensor_scalar(
                        ST[:], dlfR[:], iotaPsb[:, 0:1], None, op0=ALU.is_equal)

                    adE = pade.tile([128, k * nh], F32, tag="a")
                    for j in range(k):
                        w = int(blk_win[b0 + j])
                        if layer == 1:
                            rhs = adsb16[:, w * H:(w + 1) * H]
                        else:
                            rhs = adsb2_16[:, w:w + 1]
                        nc.tensor.matmul(adE[:, j * nh:(j + 1) * nh],
                                         lhsT=ST[:, j * 128:(j + 1) * 128],
                                         rhs=rhs, start=True, stop=True,
                                         skip_group_check=True)

                    e = pe_.tile([128, k * nh], F32, tag="e")
                    nc.vector.tensor_tensor(
                        e[:].rearrange("p (k h) -> p k h", k=k),
                        X[:, :, nx:nx + nh],
                        adE[:].rearrange("p (k h) -> p k h", k=k),
                        op=ALU.add)
                    nc.vector.scalar_tensor_tensor(
                        e[:], e[:], SLOPE, e[:], op0=ALU.mult, op1=ALU.max)
                    nc.scalar.activation(X[:, :, nx:nx + nh],
                                         e[:].rearrange("p (k h) -> p k h", k=k),
                                         AF.Exp)
                    xa = X[:].ap
                    exb = _bcast(X[:, :, nx:nx + nh],
                                 [xa[0], [tblW, k], [1, nh], [0, HID]])
                    nc.vector.tensor_tensor(
                        X[:, :, 0:nx].rearrange("p k (h c) -> p k h c", h=nh),
                        X[:, :, 0:nx].rearrange("p k (h c) -> p k h c", h=nh),
                        exb, op=ALU.mult)

                    for j in range(k):
                        b = b0 + j
                        w = int(blk_win[b])
                        if b == wb0[w]:
                            pw[w] = ppw.tile([128, ncol], F32, name=f"pw{w}", tag="w")
                        nc.tensor.matmul(pw[w][:], lhsT=S[:, j * 128:(j + 1) * 128],
                                         rhs=X[:, j, 0:ncol],
                                         start=(b == wb0[w]), stop=(b == wbl[w]),
                                         skip_group_check=True)

                    # ---- evacuate windows finishing in this chunk ----
                    for w in range(W):
                        if not (b0 <= wbl[w] < b0 + k):
                            continue
                        rden = pef.tile([128, nh], F32, tag="rd")
                        nc.vector.tensor_scalar_max(rden[:], pw[w][:, nx:nx + nh], 1e-6)
                        r = pef.tile([128, nh], F32, tag="r")
                        nc.vector.reciprocal(r[:], rden[:])
                        u = pef.tile([128, nx], F32, tag="u")
                        ra = r[:].ap
                        nc.vector.tensor_tensor(
                            u[:].rearrange("p (h c) -> p h c", h=nh),
                            pw[w][:, 0:nx].rearrange("p (h c) -> p h c", h=nh),
                            _bcast(r[:], [ra[0], [1, nh], [0, HID]]),
                            op=ALU.mult)
                        BvR = B0R if layer == 1 else B2R
                        nc.vector.tensor_add(u[:], u[:], BvR[:, 0:nx])
                        ru = pef.tile([128, nx], F32, tag="ru")
                        nc.scalar.activation(ru[:], u[:], AF.Relu)
                        pex = pef.tile([128, nx], F32, tag="pe")
                        nc.scalar.activation(pex[:], u[:], AF.Relu, scale=-1.0)
                        nc.scalar.activation(pex[:], pex[:], AF.Exp, scale=-1.0)

                        if layer == 1:
                            h1c = pef.tile([128, nx], F16, tag="h1")
                            nc.vector.tensor_add(h1c[:], ru[:], pex[:])
                            tp = pev.tile([128, HH], F16, tag="tp")
                            nc.tensor.transpose(tp[:], h1c[:], identsb[:])
                            h1T = pef.tile([128, HH], F16, tag="ht")
                            nc.any.tensor_copy(h1T[:], tp[:])
                            x2 = pev.tile([128, TW2], F32, tag="x2")
                            nc.tensor.matmul(x2[:], lhsT=h1T[:], rhs=w2e16[:],
                                             start=True, stop=True,
                                             skip_group_check=True)
                            sh2 = pef.tile([128, TW2], F16, tag="s2")
                            nc.vector.tensor_sub(sh2[:], x2[:], csum2R[:])
                            nc.vector.tensor_copy(adsb2_16[:, w:w + 1],
                                                  sh2[:, HID + 1:HID + 2])
                            dd = nc.sync.dma_start(t2in[w * 128:(w + 1) * 128, :],
                                                   sh2[:])
                            t2_dmas.append(dd)
                        else:
                            h2e = pef.tile([128, HID + 1], F16, tag="h2")
                            nc.vector.tensor_add(h2e[:, 0:HID], ru[:], pex[:])
                            nc.vector.memset(h2e[:, HID:HID + 1], 1.0)
                            Gt = pef.tile([128, G], F16, tag="g")
                            gga = glsb[:].ap
                            nc.vector.tensor_tensor(
                                Gt[:],
                                _bcast(glsb[:, w:w + 1], [gga[0], [0, G]]),
                                iotaGR[:], op=ALU.is_equal)
                            nc.tensor.matmul(poolps[:], lhsT=h2e[:], rhs=Gt[:],
                                             start=(w == 0), stop=(w == W - 1),
                                             skip_group_check=True)
            return

        edge_layer(1)

        # ============ phase 4: AllGather table2 ============
        cc1 = nc.gpsimd.collective_compute(
            "AllGather", ALU.bypass, replica_groups=rg,
            ins=[t2in[:, :]], outs=[table2[:, :]])
        for dd in t2_dmas:
            tile.add_dep_helper(cc1.ins, dd.ins, sync=True)
        t2_dmas = [cc1]

        # ============ phase 5: L2 edge loop + pooling ============
        with tc.tile_pool(name="poolpsp", bufs=1, space="PSUM") as ppool:
            poolps = ppool.tile([HID + 1, G], F32)
            edge_layer(2, poolps=poolps)

            # ============ phase 6: finale ============
            poolsb = cp.tile([HID + 1, G], F32)
            nc.any.tensor_copy(poolsb[:], poolps[:])
        dpi = nc.sync.dma_start(poolin[:, :], poolsb[:])
        cc2 = nc.gpsimd.collective_compute(
            "AllReduce", ALU.add, replica_groups=rg,
            ins=[poolin[:, :]], outs=[poolout[:, :]])
        tile.add_dep_helper(cc2.ins, dpi.ins, sync=True)
        psb = cp.tile([HID + 1, G], F32)
        d2 = nc.sync.dma_start(psb[:], poolout[:, :])
        tile.add_dep_helper(d2.ins, cc2.ins, sync=True)
        cntt = cp.tile([1, G], F32)
        nc.vector.tensor_scalar_max(cntt[:], psb[HID:HID + 1, :], 1.0)
        rec = cp.tile([1, G], F32)
        nc.vector.reciprocal(rec[:], cntt[:])
        nmk = cp.tile([1, G], F32)
        nc.vector.tensor_scalar(nmk[:], psb[HID:HID + 1, :], 0.5, None, op0=ALU.is_lt)
        drc = nc.sync.dma_start(recd[:, :], rec[:])
        dnm = nc.sync.dma_start(nmd[:, :], nmk[:])
        recR = cp.tile([HID, G], F32)
        d3 = nc.sync.dma_start(out=recR[:], in_=_bcast(recd[:, :], [[0, HID], [1, G]]))
        tile.add_dep_helper(d3.ins, drc.ins, sync=True)
        nmR = cp.tile([HID, G], F32)
        d4 = nc.sync.dma_start(out=nmR[:], in_=_bcast(nmd[:, :], [[0, HID], [1, G]]))
        tile.add_dep_helper(d4.ins, dnm.ins, sync=True)
        mean0 = cp.tile([HID, G], F32)
        nc.vector.tensor_tensor(mean0[:], psb[0:HID, :], recR[:], op=ALU.mult)
        meanc = cp.tile([HID, G], F16)
        nc.vector.tensor_tensor(meanc[:], mean0[:], nmR[:], op=ALU.add)
        with tc.tile_pool(name="fin_ps", bufs=1, space="PSUM") as pfin:
            finps = pfin.tile([OUT, G], F32)
            nc.tensor.matmul(finps[:], lhsT=wc16[:], rhs=meanc[:], start=True, stop=True)
            osb = cp.tile([OUT, G], F32)
            nc.vector.tensor_scalar_add(osb[:], finps[:], biasc[:, 0:1])
        nc.sync.dma_start(outT[:, :], osb[:])

    nc.finalize()
    return nc


# --------------------------------------------------------------------------
# host entry point
# --------------------------------------------------------------------------
def make_in_maps(inputs, P, n_cores):
    N = inputs["x"].shape[0]
    NPAD, NP, SH = P["NPAD"], P["NP"], P["SH"]
    x = np.asarray(inputs["x"], np.float32)
    TS = P["TS"]
    Wc = np.asarray(inputs["Wc"], np.float32)
    common = {
        "W1": np.asarray(inputs["W1"], np.float32),
        "as1": np.asarray(inputs["att_src1"], np.float32),
        "ad1": np.asarray(inputs["att_dst1"], np.float32),
        "g1": np.asarray(inputs["gamma1"], np.float32).reshape(1, -1),
        "be1": np.asarray(inputs["beta1"], np.float32).reshape(1, -1),
        "rm1": np.asarray(inputs["rm1"], np.float32).reshape(1, -1),
        "rv1": np.asarray(inputs["rv1"], np.float32).reshape(1, -1),
        "bb1": np.asarray(inputs["b1"], np.float32).reshape(1, -1),
        "W2": np.asarray(inputs["W2"], np.float32),
        "as2": np.asarray(inputs["att_src2"], np.float32),
        "ad2": np.asarray(inputs["att_dst2"], np.float32),
        "g2": np.asarray(inputs["gamma2"], np.float32).reshape(1, -1),
        "be2": np.asarray(inputs["beta2"], np.float32).reshape(1, -1),
        "rm2": np.asarray(inputs["rm2"], np.float32).reshape(1, -1),
        "rv2": np.asarray(inputs["rv2"], np.float32).reshape(1, -1),
        "bb2": np.asarray(inputs["b2"], np.float32).reshape(1, -1),
        "Wc": Wc,
        "WcT": np.ascontiguousarray(Wc.T),
        "bcT": np.asarray(inputs["bc"], np.float32).reshape(-1, 1),
        "iota128": np.arange(128, dtype=np.float16).reshape(1, 128),
        "iotaG": np.arange(G, dtype=np.float16).reshape(1, G),
        "iotaPf": np.arange(128, dtype=np.float32).reshape(128, 1),
    }
    in_maps = []
    for c in range(n_cores):
        m = dict(common)
        xT2 = np.zeros((IN, NP), np.float32)
        hi = min(N, c * SH + NP)
        xT2[:, :hi - c * SH] = x[c * SH:hi].T
        m["xT2"] = xT2
        xTs = np.zeros((IN, TS * 128), np.float32)
        lo2 = min(N, c * TS * 128)
        hi2 = min(N, (c + 1) * TS * 128)
        if hi2 > lo2:
            xTs[:, :hi2 - lo2] = x[lo2:hi2].T
        m["xTs"] = xTs
        for nm in ["srcP", "src2P", "dlP", "dlF", "glP"]:
            m[nm] = P[nm][c]
        in_maps.append(m)
    return in_maps


def kernel(**inputs):
    ei = np.asarray(inputs["edge_index"]).astype(np.int64)
    batch = np.asarray(inputs["batch"]).astype(np.int64)
    N = inputs["x"].shape[0]
    P = _prep(ei, batch, N, NC)
    meta = dict(P)
    meta["n_cores"] = NC
    nc = _build(meta)
    in_maps = make_in_maps(inputs, P, NC)
    import time as _time
    t0 = _time.monotonic()
    res = run_bass_kernel_spmd(nc, in_maps, core_ids=list(range(NC)))
    globals()["LAST_RUN_S"] = _time.monotonic() - t0
    outT = np.asarray(res.results[0]["outT"])
    return np.ascontiguousarray(outT.T).astype(np.float32)
